# revision 1
# baseline (speedup 1.0000x reference)
"""AdaAttN-style attention kernel for Trainium2, SPMD over 8 NeuronCores.

Math (C=256, N=8192, HW=96*96=9216):
    qn  = instnorm(q.T)                 # (C, N), stats over N
    qe  = qw @ qn + qb                  # (C, N)
    kn  = instnorm(st),  st = k[0]      # (C, HW), stats over HW
    ke  = kw @ kn + kb                  # (C, HW)
    se  = (sw @ st + sb).T              # (HW, C)
    A   = softmax(qe.T @ ke / sqrt(C))  # (N, HW)
    mean = A @ se; var = relu(A @ se^2 - mean^2)
    out = qn.T * sqrt(var) + mean       # (N, C)

Sharding: rows (N) split across the 8 cores; style side (k, weights) is
replicated and recomputed per core.  The q instance-norm statistics are the
only global coupling: shard-local bn_stats + a tiny gpsimd AllReduce of
[m_i, v_i + m_i^2].

fp8 (TRN e4m3, max 240) everywhere on the attention path, with DoubleRow
matmuls (2 contraction rows per cycle -> 2x PE throughput; the C=256 /
HW-pair contractions map exactly onto the [128, 2, free] k-tile layout):
  - st, qwT, swT ship from host pre-cast to fp8; kwT ships bf16 and is
    row-scaled by rs_k into fp8 on device (fold of the style instance norm).
  - The q-side instance norm folds into the DATA instead of the weights:
    qe = qw @ (rs_q * q) + (qb - qw @ (rs_q * m_q)), so qwT stays raw fp8
    and the fp32->fp8 cast of the q shard applies rs_q for free.
  - exp(SCALE*logit - 5.5) is written straight to fp8 by the ACT engine; the
    e^-5.5 factor cancels between softmax numerator and denominator, and
    keeps E <= ~60 < 240 (fp8 inf threshold).  max |SCALE*logit| ~ 9.6.
  - The softmax denominator is a per-pair ones-matmul on the PE (rank-1
    DoubleRow, out [1,512]) accumulated in a PSUM bank - the 9.4M-element
    reduction costs ~15us of PE instead of ~77us of DVE.
  - W2 holds [se | se^2] fp8 per hw tile; sb folds into the epilogue.
Final combine (qn, mean, std) stays fp32.  Softmax runs without
max-subtraction (logits bounded ~|10|).  PSUM: 4 mv accumulators + 3 logits
banks + 1 denominator bank = 8.
"""

import sys

if "/opt/trn_rl_repo" not in sys.path:
    sys.path.insert(0, "/opt/trn_rl_repo")

import numpy as np
import ml_dtypes

_f8_np = ml_dtypes.float8_e4m3
_bf16_np = ml_dtypes.bfloat16

C = 256
N = 8192
HW = 96 * 96  # 9216
NCORES = 8
NSH = N // NCORES  # 1024 rows per core
EPS = 1e-5
SCALE = C**-0.5
ESHIFT = 5.5  # exp(scale*logit - ESHIFT); cancels in softmax, keeps E < 240

CT = C // 128  # 2 channel tiles
HT = HW // 128  # 72 hw tiles
HP = HT // 2  # 36 hw tile pairs
NG = NSH // 512  # 2 n-groups per core
NJ = 4  # 4 row subtiles (128) per group
SCH = 1024  # st dma chunk (free dim)


def build(sim_local=False):
    import contextlib

    import concourse.bacc as bacc
    import concourse.bass as bass
    import concourse.tile as tile
    from concourse import mybir
    from concourse.masks import make_identity

    fp32 = mybir.dt.float32
    bf16 = mybir.dt.bfloat16
    f8 = mybir.dt.float8e4
    AF = mybir.ActivationFunctionType
    ALU = mybir.AluOpType
    DR = mybir.MatmulPerfMode.DoubleRow

    nc = bacc.Bacc()

    qT_sh = nc.dram_tensor("qT_sh", [C, NSH], fp32, kind="ExternalInput")
    st_d = nc.dram_tensor("st", [C, HW], f8, kind="ExternalInput")
    qwT_d = nc.dram_tensor("qwT", [C, C], f8, kind="ExternalInput")
    kwT_d = nc.dram_tensor("kwT", [C, C], bf16, kind="ExternalInput")
    swT_d = nc.dram_tensor("swT", [C, C], f8, kind="ExternalInput")
    qb_d = nc.dram_tensor("qb", [C], fp32, kind="ExternalInput")
    kb_d = nc.dram_tensor("kb", [C], fp32, kind="ExternalInput")
    sb_d = nc.dram_tensor("sb", [C], fp32, kind="ExternalInput")
    out_d = nc.dram_tensor("out", [NSH, C], fp32, kind="ExternalOutput")
    cc_in = nc.dram_tensor("cc_in", [C, 2], fp32)
    cc_out = nc.dram_tensor("cc_out", [C, 2], fp32, addr_space="Shared")

    def bcast128(ap1d):
        return bass.AP(tensor=ap1d.tensor, offset=ap1d.offset, ap=[[0, 128], ap1d.ap[0]])

    with tile.TileContext(nc) as tc, contextlib.ExitStack() as ctx:
        consts = ctx.enter_context(tc.tile_pool(name="consts", bufs=1))
        kside = ctx.enter_context(tc.tile_pool(name="kside", bufs=1))

        ident = consts.tile([128, 128], fp32)
        ones_f = consts.tile([128, 1], fp32)
        ones8 = consts.tile([128, CT, 32], f8)  # DoubleRow ldweights needs M>=32
        eps_t = consts.tile([128, 1], fp32)
        nsh_t = consts.tile([128, 1], fp32)  # -ESHIFT for the exp bias
        sb_bc = consts.tile([128, C], fp32)
        qb_sb = consts.tile([128, CT], fp32)
        kb_sb = consts.tile([128, CT], fp32)

        def emit_consts():
            # emitted after the q-stats chain so the shard DMAs and the
            # collective sit first in the DMA/gpsimd queues
            make_identity(nc, ident)
            nc.vector.memset(ones_f, 1.0)
            nc.vector.memset(ones8, 1.0)
            nc.vector.memset(eps_t, EPS)
            nc.vector.memset(nsh_t, -ESHIFT)
            nc.sync.dma_start(out=sb_bc, in_=bcast128(sb_d[:]))
            for co in range(CT):
                nc.sync.dma_start(
                    out=qb_sb[:, co : co + 1],
                    in_=qb_d[co * 128 : (co + 1) * 128].rearrange("(p o) -> p o", o=1),
                )
                nc.sync.dma_start(
                    out=kb_sb[:, co : co + 1],
                    in_=kb_d[co * 128 : (co + 1) * 128].rearrange("(p o) -> p o", o=1),
                )

        # ---- resident tensors (live into the main loop) ----
        st8 = kside.tile([128, CT, HW], f8)  # 18KB/part
        ke8 = kside.tile([128, CT, HW], f8)  # 18KB/part
        W2 = kside.tile([128, HP, 2, 512], f8)  # [se | se^2] per hw tile, 36KB/part
        qeT8 = kside.tile([128, CT, NSH], f8)
        qn_nat = kside.tile([128, NSH // 128, C], fp32)  # (n%128, n//128, c)

        stat_q = kside.tile([128, CT, NSH // 512, 6], fp32)
        stat_k = kside.tile([128, CT, HW // 512, 6], fp32)
        mv_q = kside.tile([128, CT, 2], fp32)
        mv_k = kside.tile([128, CT, 2], fp32)
        rs_q = kside.tile([128, CT], fp32)
        rs_k = kside.tile([128, CT], fp32)
        mqs8 = kside.tile([128, CT, 1], f8)  # rs_q * m_q, fp8
        mk8 = kside.tile([128, CT, 1], f8)  # m_k, fp8
        lntmp = kside.tile([128, CT], fp32)
        qwT8 = kside.tile([128, CT, C], f8)
        kwTs8 = kside.tile([128, CT, C], f8)  # kwT rows * rs_k
        swT8 = kside.tile([128, CT, C], f8)
        biasq = kside.tile([128, CT], fp32)
        biask = kside.tile([128, CT], fp32)

        with tc.tile_pool(name="setup", bufs=2) as setup, tc.tile_pool(
            name="ps_setup", bufs=3, space="PSUM"
        ) as ps_setup, tc.tile_pool(name="ps_small", bufs=1, space="PSUM") as ps_small:
            # ---- q stats: shard-local bn_stats + cross-core AllReduce ----
            qsh_f = setup.tile([128, CT, NSH], fp32, name="qsh_f", bufs=1)
            mv_loc = setup.tile([128, CT, 2], fp32, name="mv_loc", bufs=1)
            part = setup.tile([128, CT, 2], fp32, name="part", bufs=1)
            red = setup.tile([128, CT, 2], fp32, name="red", bufs=1)
            for ci in range(CT):
                for s in range(NSH // 512):
                    nc.sync.dma_start(
                        out=qsh_f[:, ci, s * 512 : (s + 1) * 512],
                        in_=qT_sh[ci * 128 : (ci + 1) * 128, s * 512 : (s + 1) * 512],
                    )
                    nc.vector.bn_stats(
                        out=stat_q[:, ci, s, :],
                        in_=qsh_f[:, ci, s * 512 : (s + 1) * 512],
                    )
                nc.vector.bn_aggr(out=mv_loc[:, ci, :], in_=stat_q[:, ci])
                # part = [m_i, v_i + m_i^2]
                nc.vector.tensor_mul(
                    out=part[:, ci, 0:1], in0=mv_loc[:, ci, 0:1], in1=mv_loc[:, ci, 0:1]
                )
                nc.vector.tensor_add(
                    out=part[:, ci, 1:2], in0=mv_loc[:, ci, 1:2], in1=part[:, ci, 0:1]
                )
                nc.vector.tensor_copy(out=part[:, ci, 0:1], in_=mv_loc[:, ci, 0:1])
                nc.sync.dma_start(
                    out=cc_in[ci * 128 : (ci + 1) * 128, :], in_=part[:, ci, :]
                )
            if sim_local:
                # CoreSim can't run the 8-core collective: fake it with a
                # DRAM->DRAM copy (stats become shard-local; the sim harness
                # compares against a shard-local-stats reference).
                nc.sync.dma_start(out=cc_out[:], in_=cc_in[:])
            else:
                nc.gpsimd.collective_compute(
                    "AllReduce",
                    ALU.add,
                    replica_groups=[list(range(NCORES))],
                    ins=[cc_in[:]],
                    outs=[cc_out[:]],
                )
            emit_consts()

            # ---- style side ships fp8; stats straight off the fp8 copy ----
            for ci in range(CT):
                nc.sync.dma_start(out=swT8[:, ci, :], in_=swT_d[ci * 128 : (ci + 1) * 128, :])
                nc.sync.dma_start(out=qwT8[:, ci, :], in_=qwT_d[ci * 128 : (ci + 1) * 128, :])
            for ci in range(CT):
                for ch in range(HW // SCH):
                    nc.sync.dma_start(
                        out=st8[:, ci, ch * SCH : (ch + 1) * SCH],
                        in_=st_d[ci * 128 : (ci + 1) * 128, ch * SCH : (ch + 1) * SCH],
                    )
                    for s in range(SCH // 512):
                        nc.vector.bn_stats(
                            out=stat_k[:, ci, ch * (SCH // 512) + s, :],
                            in_=st8[:, ci, ch * SCH + s * 512 : ch * SCH + (s + 1) * 512],
                        )
                nc.vector.bn_aggr(out=mv_k[:, ci, :], in_=stat_k[:, ci])

            # ---- rs_k = exp(-0.5*ln(v+eps)); fold kw rows; mk -> fp8 ----
            for ci in range(CT):
                nc.scalar.activation(
                    out=lntmp[:, ci : ci + 1], in_=mv_k[:, ci, 1:2], func=AF.Ln, bias=eps_t
                )
                nc.scalar.activation(
                    out=rs_k[:, ci : ci + 1], in_=lntmp[:, ci : ci + 1], func=AF.Exp, scale=-0.5
                )
                nc.vector.tensor_copy(out=mk8[:, ci, :], in_=mv_k[:, ci, 0:1])
            for ci in range(CT):
                kwtmp = setup.tile([128, C], bf16, name="kwtmp", bufs=1)
                nc.sync.dma_start(out=kwtmp, in_=kwT_d[ci * 128 : (ci + 1) * 128, :])
                nc.vector.tensor_scalar_mul(
                    out=kwTs8[:, ci, :], in0=kwtmp, scalar1=rs_k[:, ci : ci + 1]
                )

            # ---- ke = kw' @ st + biask  (fp8, (C, HW)); DoubleRow mms ----
            # biask = kb - kw' @ mk  first (k-side only; not collective-gated)
            bk_ps = ps_small.tile([128, CT], fp32, name="bk_ps")
            for co in range(CT):
                nc.tensor.matmul(
                    bk_ps[:, co : co + 1],
                    kwTs8[:, :, co * 128 : (co + 1) * 128],
                    mk8,
                    start=(co == 0),
                    stop=(co == CT - 1),
                    skip_group_check=True,
                    perf_mode=DR,
                )
            for co in range(CT):
                nc.vector.tensor_sub(
                    out=biask[:, co : co + 1], in0=kb_sb[:, co : co + 1], in1=bk_ps[:, co : co + 1]
                )
            # ---- ke = kw' @ st + biask and W2 = [se | se^2], interleaved so
            # production order matches main-loop consumption; ACT stays
            # exp/ln-only so the activation table never reloads mid-kernel.
            # Bias adds on DVE, se copies on DVE, squares on Pool from the
            # SBUF fp8 copy (gpsimd cannot touch PSUM).
            for ch in range(HW // 512):
                for co in range(CT):
                    ke_ps = ps_setup.tile([128, 512], fp32, name="ke_ps", tag="mm_ps")
                    nc.tensor.matmul(
                        ke_ps,
                        kwTs8[:, :, co * 128 : (co + 1) * 128],
                        st8[:, :, ch * 512 : (ch + 1) * 512],
                        start=True,
                        stop=True,
                        perf_mode=DR,
                    )
                    nc.vector.tensor_scalar_add(
                        out=ke8[:, co, ch * 512 : (ch + 1) * 512],
                        in0=ke_ps,
                        scalar1=biask[:, co : co + 1],
                    )
                for t in range(2 * ch, 2 * ch + 2):
                    se_ps = ps_setup.tile([128, 2, C], fp32, name="se_ps", tag="mm_ps")
                    for half in range(2):
                        h = 2 * t + half
                        nc.tensor.matmul(
                            se_ps[:, half, :],
                            st8[:, :, h * 128 : (h + 1) * 128],
                            swT8,
                            start=True,
                            stop=True,
                            perf_mode=DR,
                        )
                    nc.vector.tensor_copy(out=W2[:, t, :, 0:256], in_=se_ps)
                    nc.gpsimd.tensor_mul(
                        out=W2[:, t, :, 256:512],
                        in0=W2[:, t, :, 0:256],
                        in1=W2[:, t, :, 0:256],
                    )

            # ---- q side (post-collective): global stats -> rs_q ----
            for ci in range(CT):
                nc.sync.dma_start(
                    out=red[:, ci, :], in_=cc_out[ci * 128 : (ci + 1) * 128, :]
                )
                inv_n = 1.0 if sim_local else 1.0 / NCORES
                nc.vector.tensor_scalar_mul(
                    out=mv_q[:, ci, 0:1], in0=red[:, ci, 0:1], scalar1=inv_n
                )
                nc.vector.tensor_scalar_mul(
                    out=mv_q[:, ci, 1:2], in0=red[:, ci, 1:2], scalar1=inv_n
                )
                nc.vector.tensor_mul(
                    out=red[:, ci, 0:1], in0=mv_q[:, ci, 0:1], in1=mv_q[:, ci, 0:1]
                )
                nc.vector.tensor_sub(
                    out=mv_q[:, ci, 1:2], in0=mv_q[:, ci, 1:2], in1=red[:, ci, 0:1]
                )
                nc.scalar.activation(
                    out=lntmp[:, ci : ci + 1], in_=mv_q[:, ci, 1:2], func=AF.Ln, bias=eps_t
                )
                nc.scalar.activation(
                    out=rs_q[:, ci : ci + 1], in_=lntmp[:, ci : ci + 1], func=AF.Exp, scale=-0.5
                )
                # mqs8 = rs_q * m_q in fp8 (for the folded bias matmul)
                nc.vector.tensor_mul(
                    out=red[:, ci, 1:2], in0=mv_q[:, ci, 0:1], in1=rs_q[:, ci : ci + 1]
                )
                nc.vector.tensor_copy(out=mqs8[:, ci, :], in_=red[:, ci, 1:2])

            # ---- biasq = qb - qw @ (rs_q * m_q) ----
            bq_ps = ps_small.tile([128, CT], fp32, name="bq_ps")
            for co in range(CT):
                nc.tensor.matmul(
                    bq_ps[:, co : co + 1],
                    qwT8[:, :, co * 128 : (co + 1) * 128],
                    mqs8,
                    start=(co == 0),
                    stop=(co == CT - 1),
                    skip_group_check=True,
                    perf_mode=DR,
                )
            for co in range(CT):
                nc.vector.tensor_sub(
                    out=biasq[:, co : co + 1], in0=qb_sb[:, co : co + 1], in1=bq_ps[:, co : co + 1]
                )

            # ---- q shard: fold rs_q into the fp8 cast; qn stays fp32 ----
            qsh8 = setup.tile([128, CT, NSH], f8, name="qsh8", bufs=1)
            qnT = setup.tile([128, CT, NSH], fp32, name="qnT", bufs=1)
            for ci in range(CT):
                nc.vector.tensor_scalar_mul(
                    out=qsh8[:, ci, :], in0=qsh_f[:, ci, :], scalar1=rs_q[:, ci : ci + 1]
                )
                nc.vector.tensor_scalar(
                    out=qnT[:, ci, :],
                    in0=qsh_f[:, ci, :],
                    scalar1=mv_q[:, ci, 0:1],
                    scalar2=rs_q[:, ci : ci + 1],
                    op0=ALU.subtract,
                    op1=ALU.mult,
                )

            # ---- qe = qw @ (rs_q*q) + biasq  (fp8, (C, NSH)); DoubleRow ----
            for co in range(CT):
                for nn in range(NSH // 512):
                    qe_ps = ps_setup.tile([128, 512], fp32, name="qe_ps", tag="mm_ps")
                    nc.tensor.matmul(
                        qe_ps,
                        qwT8[:, :, co * 128 : (co + 1) * 128],
                        qsh8[:, :, nn * 512 : (nn + 1) * 512],
                        start=True,
                        stop=True,
                        perf_mode=DR,
                    )
                    nc.vector.tensor_scalar_add(
                        out=qeT8[:, co, nn * 512 : (nn + 1) * 512],
                        in0=qe_ps,
                        scalar1=biasq[:, co : co + 1],
                    )

            # ---- pre-transpose qn to natural (n, c) layout (fp32 PE) ----
            for tp in range(NSH // 256):
                qt_ps = ps_setup.tile([128, 512], fp32, name="qt_ps", tag="mm_ps")
                for half in range(2):
                    t = tp * 2 + half
                    for ci in range(CT):
                        nc.tensor.transpose(
                            qt_ps[:, half * 256 + ci * 128 : half * 256 + (ci + 1) * 128],
                            qnT[:, ci, t * 128 : (t + 1) * 128],
                            ident,
                        )
                nc.vector.tensor_copy(
                    out=qn_nat[:, tp * 2 : tp * 2 + 2, :], in_=qt_ps
                )

        # ================= main loop =================
        with tc.tile_pool(name="mvps", bufs=1, space="PSUM") as mvps, tc.tile_pool(
            name="sps", bufs=1, space="PSUM"
        ) as sps, tc.tile_pool(name="lgps", bufs=3, space="PSUM") as lgps, tc.tile_pool(
            name="epool", bufs=4
        ) as epool, tc.tile_pool(name="ep", bufs=1) as ep, tc.tile_pool(
            name="outp", bufs=2
        ) as outp:
            for g in range(NG):
                mv_acc = [mvps.tile([128, 512], fp32, name=f"mv{j}") for j in range(NJ)]
                # one bank shared: rows 0..31 accumulate the denominator (the
                # ones weights are [128,2,32], so the 32 output rows are
                # identical copies of the per-n sums); after draining to SBUF
                # the same bank receives the 32x128 block transposes.
                sboth = sps.tile([128, 512], fp32, name="sboth")
                s_acc = sboth[0:32, :]

                def mm1(h):
                    lg = lgps.tile([128, 512], fp32, name="lg")
                    nc.tensor.matmul(
                        lg,
                        ke8[:, :, h * 128 : (h + 1) * 128],
                        qeT8[:, :, g * 512 : (g + 1) * 512],
                        start=True,
                        stop=True,
                        perf_mode=DR,
                    )
                    return lg

                # software pipeline, two deep: logits mms run two h ahead of
                # the values mms so exp always has a finished bank waiting.
                pend = [mm1(0), mm1(1)]
                E_pair = None
                for h in range(HT):
                    lg = pend.pop(0)
                    if h % 2 == 0:
                        E_pair = epool.tile([128, 2, 512], f8, name="E_pair")
                    nc.scalar.activation(
                        out=E_pair[:, h % 2, :], in_=lg, func=AF.Exp, scale=SCALE,
                        bias=nsh_t,
                    )
                    if h + 2 < HT:
                        pend.append(mm1(h + 2))
                    if h % 2 == 1:
                        hp = h // 2
                        # softmax denominator: rank-1 ones matmul, accumulated
                        # across pairs in a [1, 512] PSUM row
                        nc.tensor.matmul(
                            s_acc,
                            ones8,
                            E_pair,
                            start=(hp == 0),
                            stop=(hp == HP - 1),
                            perf_mode=DR,
                        )
                        for j in range(NJ):
                            nc.tensor.matmul(
                                mv_acc[j],
                                E_pair[:, :, j * 128 : (j + 1) * 128],
                                W2[:, hp, :, :],
                                start=(hp == 0),
                                stop=(hp == HP - 1),
                                perf_mode=DR,
                            )

                # ---- drain PSUM accumulators on DVE: keeps ACT exp/ln-only so
                # the activation table never reloads at group boundaries ----
                mv_sb = ep.tile([128, NJ, 512], fp32, name="mv_sb")
                for j in range(NJ):
                    nc.vector.tensor_copy(out=mv_sb[:, j, :], in_=mv_acc[j])
                # s rows -> SBUF, then 32x128 PE block transposes back into the
                # (drained) bank; column 0 of each 128-col block holds s[n]
                s_sb32 = ep.tile([32, 512], fp32, name="s_sb32")
                nc.vector.tensor_copy(out=s_sb32, in_=s_acc)
                for j in range(NJ):
                    nc.tensor.matmul(
                        sboth[:, j * 128 : j * 128 + 32],
                        s_sb32[0:32, j * 128 : (j + 1) * 128],
                        ident[0:32, 0:32],
                        is_transpose=True,
                    )
                sT_view = sboth.rearrange("p (j r) -> p j r", r=128)[:, :, 0]

                # ---- epilogue, elementwise ops batched across the 4 subtiles ----
                inv = ep.tile([128, NJ], fp32, name="inv")
                nc.vector.reciprocal(out=inv, in_=sT_view)
                mean_a = ep.tile([128, NJ, C], fp32, name="mean_a")
                var_a = ep.tile([128, NJ, C], fp32, name="var_a")
                for j in range(NJ):
                    nc.vector.tensor_scalar_mul(
                        out=mean_a[:, j, :], in0=mv_sb[:, j, 0:C], scalar1=inv[:, j : j + 1]
                    )
                    nc.vector.tensor_scalar_mul(
                        out=var_a[:, j, :], in0=mv_sb[:, j, C : 2 * C], scalar1=inv[:, j : j + 1]
                    )
                msq = ep.tile([128, NJ, C], fp32, name="msq")
                nc.vector.tensor_mul(out=msq, in0=mean_a, in1=mean_a)
                nc.vector.tensor_sub(out=var_a, in0=var_a, in1=msq)
                nc.vector.tensor_scalar_max(out=var_a, in0=var_a, scalar1=0.0)
                # std = exp(0.5*ln(var)): stays in the exp/ln ACT table set
                std_a = ep.tile([128, NJ, C], fp32, name="std_a")
                nc.scalar.activation(out=std_a, in_=var_a, func=AF.Ln)
                nc.scalar.activation(out=std_a, in_=std_a, func=AF.Exp, scale=0.5)
                # mean of (se + sb) = raw mean + sb (var is shift-invariant)
                for j in range(NJ):
                    nc.vector.tensor_add(out=mean_a[:, j, :], in0=mean_a[:, j, :], in1=sb_bc)
                cs = outp.tile([128, NJ, C], fp32, name="cs")
                nc.vector.tensor_mul(out=cs, in0=qn_nat[:, g * NJ : (g + 1) * NJ, :], in1=std_a)
                nc.vector.tensor_add(out=cs, in0=cs, in1=mean_a)
                nc.sync.dma_start(
                    out=out_d[g * 512 : (g + 1) * 512, :].rearrange(
                        "(t p) c -> p t c", p=128
                    ),
                    in_=cs,
                )

    nc.compile()
    return nc


_cache = {}


def _get_nc():
    if "nc" not in _cache:
        _cache["nc"] = build()
    return _cache["nc"]


def _to_f8(a):
    return np.clip(np.ascontiguousarray(a, np.float32), -240.0, 240.0).astype(_f8_np)


def make_in_maps(q, k, qw, qb, kw, kb, sw, sb):
    qT = np.ascontiguousarray(q.T.astype(np.float32))
    base = {
        "st": _to_f8(k.reshape(C, HW)),
        "qwT": _to_f8(qw.T),
        "kwT": np.ascontiguousarray(kw.T.astype(np.float32)).astype(_bf16_np),
        "swT": _to_f8(sw.T),
        "qb": np.ascontiguousarray(qb.astype(np.float32)),
        "kb": np.ascontiguousarray(kb.astype(np.float32)),
        "sb": np.ascontiguousarray(sb.astype(np.float32)),
    }
    return [
        {**base, "qT_sh": np.ascontiguousarray(qT[:, i * NSH : (i + 1) * NSH])}
        for i in range(NCORES)
    ]


def kernel(q, k, qw, qb, kw, kb, sw, sb):
    from concourse.bass_utils import run_bass_kernel_spmd

    q, k, qw, qb, kw, kb, sw, sb = (
        np.asarray(a) for a in (q, k, qw, qb, kw, kb, sw, sb)
    )
    nc = _get_nc()
    in_maps = make_in_maps(q, k, qw, qb, kw, kb, sw, sb)
    res = run_bass_kernel_spmd(nc, in_maps, core_ids=list(range(NCORES)))
    out = np.concatenate([res.results[i]["out"] for i in range(NCORES)], axis=0)
    return out.astype(np.float32)

